# revision 3
# baseline (speedup 1.0000x reference)
"""Trainium2 Bass kernel for ActionExpertAttention (dense transformer block).

Strategy: data-parallel over batch (16 batches -> 2 per core on 8 cores).
All matmuls run in bf16 with fp32 PSUM accumulation. The whole pipeline is
computed in "transposed" space so nothing needs an on-chip transpose except
V_new (16 small PE transposes):

  qkv^T[n, m]   = wqkv^T_chunk^T . hs^T          (n-chunks of 128)
  scores^T[k,q] = Krot^T_chunk^T . Qrot^T        (kv-chunks of 128)
  out^T[d, q]   = V_chunk^T      . exp(scores^T) (accumulated over kv)
  final[q, n]   = attn^T_chunk^T . wo^T          (accumulated over heads)

Softmax denominators come from a ones-vector matmul over exp(scores^T); the
reciprocal is exp(-ln(s)) on the ACT engine; normalization is fused into the
PSUM->SBUF evict of out^T. RoPE rotate-half is done with sign-baked sin tables
and (for the vlm K, which is loaded from HBM) a host-prepared half-swapped
copy so every DVE op is a full-width tensor_tensor.
"""

import sys

sys.path.insert(0, "/opt/trn_rl_repo")

import numpy as np
import ml_dtypes

import concourse.bass as bass
import concourse.tile as tile
from concourse import mybir, bacc
from concourse.bass_utils import run_bass_kernel_spmd
from concourse.masks import make_identity

BF = ml_dtypes.bfloat16

B, Q, VLM = 16, 128, 2048
H, HKV, D = 16, 8, 128
HID = H * D            # 2048
G = H // HKV           # 2
KV = VLM + Q           # 2176
THETA = 10000.0
N_CORES = 8
B_LOC = B // N_CORES   # 2
KDIM = HID             # contraction dim of qkv proj
NQKV = (H + 2 * HKV) * D  # 4096
KO = KDIM // 128       # 16
NCH = NQKV // 128      # 32 qkv output chunks
KVCH = KV // 128       # 17 kv chunks
M = B_LOC * Q          # 256 moving columns (both local batches)

f32 = mybir.dt.float32
bf16 = mybir.dt.bfloat16


def _build_nc():
    nc = bacc.Bacc(trn_type="TRN2")

    hsT_d = nc.dram_tensor("hsT", [KDIM, M], bf16, kind="ExternalInput")
    wqkvT_d = nc.dram_tensor("wqkvT", [KDIM, NQKV], bf16, kind="ExternalInput")
    woT_d = nc.dram_tensor("woT", [HID, HID], bf16, kind="ExternalInput")
    vkT_d = nc.dram_tensor("vkT", [B_LOC, HKV, D, VLM], bf16, kind="ExternalInput")
    vkTs_d = nc.dram_tensor("vkTs", [B_LOC, HKV, D, VLM], bf16, kind="ExternalInput")
    vv_d = nc.dram_tensor("vv", [B_LOC, HKV, VLM, D], bf16, kind="ExternalInput")
    cos_d = nc.dram_tensor("cosT", [D, KV], bf16, kind="ExternalInput")
    sins_d = nc.dram_tensor("sinTs", [D, KV], bf16, kind="ExternalInput")
    cosq_d = nc.dram_tensor("cosqT", [B_LOC, D, Q], bf16, kind="ExternalInput")
    sinq_d = nc.dram_tensor("sinqTs", [B_LOC, D, Q], bf16, kind="ExternalInput")
    maskT_d = nc.dram_tensor("maskT", [B_LOC, Q, Q], f32, kind="ExternalInput")
    out_d = nc.dram_tensor("out", [B_LOC, Q, HID], f32, kind="ExternalOutput")

    with tile.TileContext(nc) as tc:
        from contextlib import ExitStack

        with ExitStack() as ctx:
            const = ctx.enter_context(tc.tile_pool(name="const", bufs=1))
            wqp = ctx.enter_context(tc.tile_pool(name="wq", bufs=4))
            wop = ctx.enter_context(tc.tile_pool(name="wo", bufs=2))
            ktp = ctx.enter_context(tc.tile_pool(name="kt", bufs=2))
            ktsp = ctx.enter_context(tc.tile_pool(name="kts", bufs=2))
            krotp = ctx.enter_context(tc.tile_pool(name="krot", bufs=2))
            ktmpp = ctx.enter_context(tc.tile_pool(name="ktmp", bufs=2))
            vvp = ctx.enter_context(tc.tile_pool(name="vv", bufs=2))
            expp = ctx.enter_context(tc.tile_pool(name="expp", bufs=2))
            tmp = ctx.enter_context(tc.tile_pool(name="tmp", bufs=6))
            outp = ctx.enter_context(tc.tile_pool(name="outp", bufs=2))
            ps = ctx.enter_context(tc.tile_pool(name="ps", bufs=2, space="PSUM"))

            # ---------- residents ----------
            hs_sb = const.tile([128, KO, M], bf16, tag="hs")
            nc.gpsimd.dma_start(
                out=hs_sb, in_=hsT_d.ap().rearrange("(ko ki) m -> ki ko m", ki=128)
            )
            cos_sb = const.tile([128, KV], bf16, tag="cos")
            nc.gpsimd.dma_start(out=cos_sb, in_=cos_d[:])
            sins_sb = const.tile([128, KV], bf16, tag="sins")
            nc.gpsimd.dma_start(out=sins_sb, in_=sins_d[:])
            cosq_sb = const.tile([128, B_LOC, Q], bf16, tag="cosq")
            nc.gpsimd.dma_start(out=cosq_sb, in_=cosq_d.ap().rearrange("b d q -> d b q"))
            sinq_sb = const.tile([128, B_LOC, Q], bf16, tag="sinq")
            nc.gpsimd.dma_start(out=sinq_sb, in_=sinq_d.ap().rearrange("b d q -> d b q"))
            maskT_sb = const.tile([128, B_LOC, Q], f32, tag="maskT")
            nc.gpsimd.dma_start(out=maskT_sb, in_=maskT_d.ap().rearrange("b k q -> k b q"))

            qT_sb = const.tile([128, B_LOC, H, Q], bf16, tag="qT")
            knT_sb = const.tile([128, B_LOC, HKV, Q], bf16, tag="knT")
            vn_sb = const.tile([128, B_LOC, HKV, D], bf16, tag="vn")
            attnT_sb = const.tile([128, B_LOC, H, Q], bf16, tag="attnT")

            ones_sb = const.tile([128, 1], bf16, tag="ones")
            nc.vector.memset(ones_sb, 1.0)
            id_sb = const.tile([128, 128], bf16, tag="ident")
            make_identity(nc, id_sb)

            def rope_from_psum(seg, cos_ap, sins_ap, out_ap):
                """seg: [128, W] psum fp32 holding x^T (d on partitions).
                out = seg*cos + swap_halves(seg)*sins  (sins sign-baked)."""
                w = seg.shape[-1]
                tcos = tmp.tile([128, w], bf16, tag="tcos")
                nc.vector.tensor_tensor(out=tcos, in0=seg, in1=cos_ap, op=mybir.AluOpType.mult)
                tsin = tmp.tile([128, w], bf16, tag="tsin")
                nc.vector.tensor_tensor(
                    out=tsin[0:64, :], in0=seg[64:128, :], in1=sins_ap[0:64, :],
                    op=mybir.AluOpType.mult,
                )
                nc.vector.tensor_tensor(
                    out=tsin[64:128, :], in0=seg[0:64, :], in1=sins_ap[64:128, :],
                    op=mybir.AluOpType.mult,
                )
                nc.vector.tensor_tensor(out=out_ap, in0=tcos, in1=tsin, op=mybir.AluOpType.add)

            # ---------- phase 1: qkv^T projection ----------
            wq_re = wqkvT_d.ap().rearrange("(ko ki) n -> ki ko n", ki=128)
            for nch in range(NCH):
                hkv, slot = nch // 4, nch % 4
                wq = wqp.tile([128, KO, 128], bf16, tag="wq")
                nc.gpsimd.dma_start(out=wq, in_=wq_re[:, :, nch * 128:(nch + 1) * 128])
                pq = ps.tile([128, M], f32, tag="qkv")
                for ko in range(KO):
                    nc.tensor.matmul(
                        pq, wq[:, ko, :], hs_sb[:, ko, :],
                        start=(ko == 0), stop=(ko == KO - 1),
                    )
                if slot <= 1:
                    h = hkv * G + slot
                    for b in range(B_LOC):
                        rope_from_psum(
                            pq[:, b * Q:(b + 1) * Q],
                            cosq_sb[:, b, :], sinq_sb[:, b, :],
                            qT_sb[:, b, h, :],
                        )
                elif slot == 2:
                    for b in range(B_LOC):
                        rope_from_psum(
                            pq[:, b * Q:(b + 1) * Q],
                            cos_sb[:, VLM:VLM + Q], sins_sb[:, VLM:VLM + Q],
                            knT_sb[:, b, hkv, :],
                        )
                else:
                    vt = tmp.tile([128, M], bf16, tag="vt")
                    nc.scalar.activation(out=vt, in_=pq, func=mybir.ActivationFunctionType.Copy)
                    for b in range(B_LOC):
                        pvt = ps.tile([128, 128], bf16, tag="qkv")
                        nc.tensor.transpose(pvt, vt[:, b * Q:(b + 1) * Q], id_sb)
                        nc.scalar.activation(
                            out=vn_sb[:, b, hkv, :], in_=pvt,
                            func=mybir.ActivationFunctionType.Copy,
                        )

            # ---------- phase 2: attention per (b, hkv) ----------
            for b in range(B_LOC):
                for hkv in range(HKV):
                    kt = ktp.tile([128, VLM], bf16, tag="kt")
                    nc.gpsimd.dma_start(out=kt, in_=vkT_d[b, hkv])
                    kts = ktsp.tile([128, VLM], bf16, tag="kts")
                    nc.gpsimd.dma_start(out=kts, in_=vkTs_d[b, hkv])
                    vvt = vvp.tile([128, KO, D], bf16, tag="vv")
                    nc.gpsimd.dma_start(
                        out=vvt, in_=vv_d[b, hkv].rearrange("(ko ki) d -> ki ko d", ki=128)
                    )

                    krot = krotp.tile([128, VLM], bf16, tag="krot")
                    nc.vector.tensor_tensor(out=krot, in0=kt, in1=cos_sb[:, 0:VLM], op=mybir.AluOpType.mult)
                    ktmp = ktmpp.tile([128, VLM], bf16, tag="ktmp")
                    nc.vector.tensor_tensor(out=ktmp, in0=kts, in1=sins_sb[:, 0:VLM], op=mybir.AluOpType.mult)
                    nc.vector.tensor_tensor(out=krot, in0=krot, in1=ktmp, op=mybir.AluOpType.add)

                    qT_ap = qT_sb[:, b, hkv * G:(hkv + 1) * G, :]  # [128, 2, 128]
                    expT = expp.tile([128, KVCH, M], bf16, tag="expT")

                    for cc in range((KVCH + 1) // 2):  # chunk pairs
                        c0 = cc * 2
                        npair = 2 if c0 + 1 < KVCH else 1
                        pqk = ps.tile([128, 512], f32, tag="qk")
                        for half in range(npair):
                            c = c0 + half
                            lhsT = krot[:, c * 128:(c + 1) * 128] if c < VLM // 128 \
                                else knT_sb[:, b, hkv, :]
                            nc.tensor.matmul(
                                pqk[:, half * M:(half + 1) * M], lhsT, qT_ap,
                                start=True, stop=True,
                            )
                            if c == KVCH - 1:
                                mask_b = maskT_sb[:, b, :]
                                mask_bc = bass.AP(
                                    tensor=mask_b.tensor, offset=mask_b.offset,
                                    ap=[mask_b.ap[0], [0, G], mask_b.ap[1]],
                                )
                                seg = pqk[:, half * M:(half + 1) * M]
                                nc.vector.tensor_tensor(out=seg, in0=seg, in1=mask_bc, op=mybir.AluOpType.add)
                        nc.scalar.activation(
                            out=expT[:, c0:c0 + npair, :], in_=pqk[:, 0:npair * M],
                            func=mybir.ActivationFunctionType.Exp,
                        )

                    po = ps.tile([128, M], f32, tag="pv")
                    psum_s = ps.tile([1, M], f32, tag="sum")
                    for c in range(KVCH):
                        lhsT = vvt[:, c, :] if c < VLM // 128 else vn_sb[:, b, hkv, :]
                        nc.tensor.matmul(po, lhsT, expT[:, c, :], start=(c == 0), stop=(c == KVCH - 1))
                    for c in range(KVCH):
                        nc.tensor.matmul(psum_s, ones_sb, expT[:, c, :], start=(c == 0), stop=(c == KVCH - 1))

                    lns = tmp.tile([1, M], f32, tag="lns")
                    nc.scalar.activation(out=lns, in_=psum_s, func=mybir.ActivationFunctionType.Ln)
                    rec = tmp.tile([1, M], f32, tag="rec")
                    nc.scalar.activation(out=rec, in_=lns, func=mybir.ActivationFunctionType.Exp, scale=-1.0)
                    rec_bc = bass.AP(tensor=rec.tensor, offset=rec.offset,
                                     ap=[rec.ap[0], [0, 128], rec.ap[1]])
                    rec128 = tmp.tile([128, M], f32, tag="rec128")
                    nc.gpsimd.dma_start(out=rec128, in_=rec_bc)
                    nc.vector.tensor_tensor(
                        out=attnT_sb[:, b, hkv * G:(hkv + 1) * G, :],
                        in0=po, in1=rec128, op=mybir.AluOpType.mult,
                    )

            # ---------- phase 3: output projection ----------
            wo_re = woT_d.ap().rearrange("(h d) n -> d h n", d=128)
            NT = HID // 512
            for nt in range(NT):
                wo_t = wop.tile([128, H, 512], bf16, tag="wo")
                nc.gpsimd.dma_start(out=wo_t, in_=wo_re[:, :, nt * 512:(nt + 1) * 512])
                for b in range(B_LOC):
                    pw = ps.tile([128, 512], f32, tag="qk")
                    for h in range(H):
                        nc.tensor.matmul(
                            pw, attnT_sb[:, b, h, :], wo_t[:, h, :],
                            start=(h == 0), stop=(h == H - 1),
                        )
                    ot = outp.tile([128, 512], f32, tag="ot")
                    nc.scalar.activation(out=ot, in_=pw, func=mybir.ActivationFunctionType.Copy)
                    nc.gpsimd.dma_start(out=out_d[b, :, nt * 512:(nt + 1) * 512], in_=ot)

    nc.finalize()
    return nc


_NC_CACHE = None


def _get_nc():
    global _NC_CACHE
    if _NC_CACHE is None:
        _NC_CACHE = _build_nc()
    return _NC_CACHE


def _host_prep(hidden_states, vlm_key, vlm_value, position_ids, attention_mask,
               wqkv_w, wo_w):
    hs = np.asarray(hidden_states, dtype=np.float32)
    vk = np.asarray(vlm_key, dtype=np.float32)
    vv = np.asarray(vlm_value, dtype=np.float32)
    pos = np.asarray(position_ids).astype(np.int64)
    am = np.asarray(attention_mask, dtype=np.float32)
    wqkv = np.asarray(wqkv_w, dtype=np.float32)
    wo = np.asarray(wo_w, dtype=np.float32)

    wqkvT = np.ascontiguousarray(wqkv.T).astype(BF)
    woT = np.ascontiguousarray(wo.T).astype(BF)

    inv = 1.0 / (THETA ** (np.arange(0, D, 2, dtype=np.float32) / D))
    t = np.arange(KV, dtype=np.float32)
    fr = np.outer(t, inv)
    emb = np.concatenate([fr, fr], axis=-1)          # (KV, D)
    cosT = np.ascontiguousarray(np.cos(emb).T)       # (D, KV) fp32
    sinT = np.ascontiguousarray(np.sin(emb).T)
    sinTs = sinT.copy()
    sinTs[: D // 2] *= -1.0                          # sign baked for rotate-half
    scale = 1.0 / np.sqrt(np.float32(D))

    in_maps = []
    for core in range(N_CORES):
        bs = slice(core * B_LOC, (core + 1) * B_LOC)
        hsT_i = np.ascontiguousarray(hs[bs].transpose(2, 0, 1).reshape(KDIM, M)).astype(BF)
        vkT_i = np.ascontiguousarray(vk[bs].transpose(0, 1, 3, 2)).astype(BF)
        vkTs_i = np.ascontiguousarray(
            np.concatenate([vkT_i[:, :, D // 2:, :], vkT_i[:, :, : D // 2, :]], axis=2)
        )
        vv_i = np.ascontiguousarray(vv[bs]).astype(BF)
        posq = pos[bs] + KV - Q                       # (B_LOC, Q)
        cosq_i = np.ascontiguousarray((cosT[:, posq] * scale).transpose(1, 0, 2)).astype(BF)
        sinq_i = np.ascontiguousarray((sinTs[:, posq] * scale).transpose(1, 0, 2)).astype(BF)
        maskT_i = np.ascontiguousarray(
            np.maximum(am[bs, 0, :, VLM:], -30.0).transpose(0, 2, 1)
        ).astype(np.float32)
        in_maps.append({
            "hsT": hsT_i,
            "wqkvT": wqkvT,
            "woT": woT,
            "vkT": vkT_i,
            "vkTs": vkTs_i,
            "vv": vv_i,
            "cosT": cosT.astype(BF),
            "sinTs": sinTs.astype(BF),
            "cosqT": cosq_i,
            "sinqTs": sinq_i,
            "maskT": maskT_i,
        })
    return in_maps


def kernel(hidden_states, vlm_key, vlm_value, position_ids, attention_mask,
           wqkv_w, wo_w, _trace=False):
    nc = _get_nc()
    in_maps = _host_prep(hidden_states, vlm_key, vlm_value, position_ids,
                         attention_mask, wqkv_w, wo_w)
    res = run_bass_kernel_spmd(nc, in_maps, core_ids=list(range(N_CORES)), trace=_trace)
    out = np.concatenate([res.results[i]["out"] for i in range(N_CORES)], axis=0)
    if _trace:
        kernel._last_results = res
    return out.astype(np.float32)


if __name__ == "__main__":
    # quick self-run with random data
    rng = np.random.default_rng(0)
    ins = {
        "hidden_states": rng.standard_normal((B, Q, HID), dtype=np.float32),
        "vlm_key": rng.standard_normal((B, HKV, VLM, D), dtype=np.float32),
        "vlm_value": rng.standard_normal((B, HKV, VLM, D), dtype=np.float32),
        "position_ids": np.tile(np.arange(Q, dtype=np.int32), (B, 1)),
        "attention_mask": np.zeros((B, 1, Q, KV), dtype=np.float32),
        "wqkv_w": rng.standard_normal((NQKV, HID), dtype=np.float32) * 0.02,
        "wo_w": rng.standard_normal((HID, HID), dtype=np.float32) * 0.02,
    }
    out = kernel(**ins)
    print("out", out.shape, out.dtype, float(np.abs(out).max()))
